# revision 15
# baseline (speedup 1.0000x reference)
# BitNet GQA attention block on 8 Trainium2 NeuronCores.
#
# Sharding: data parallel over sequence (256 tokens/core). K/V are computed
# per-core for the local tokens, RoPE'd, then AllGathered so every core can
# run full (non-causal) attention for its own query tokens. Projections run
# as integer-exact bf16 matmuls (8-bit quantized activations are integers
# <=127, ternary weights are -1/0/1 -- both exact in bf16; PSUM accumulates
# in fp32 and |dot| < 2^24 so results are exact). Attention matmuls use
# float32r (full PE rate at free-dim >= 256).
import math

import numpy as np

import concourse.bacc as bacc
import concourse.bass as bass
import concourse.bass_isa as bass_isa
import concourse.mybir as mybir
import concourse.tile as tile

DT = mybir.dt
AF = mybir.ActivationFunctionType
ALU = mybir.AluOpType
AX = mybir.AxisListType

H = 4096
QH, KVH, D = 32, 8, 128     # query heads, kv heads, head dim
HB = H // 128               # 32 hidden blocks
NREP = QH // KVH
ROUND_C = 12582912.0        # 1.5 * 2**23: fp32 add forces round-to-nearest-even int
LN_EPS = 1e-5
QB = 127.0
SM_SCALE = 1.0 / math.sqrt(128.0)


def build(n_cores=8, s_pc=256):
    """Build the SPMD Bass program (identical on all cores; per-core data via inputs)."""
    NT = s_pc // 128            # token tiles per core
    S = s_pc * n_cores
    KT = S // 128               # key-token tiles after gather
    f32, bf16, f32r = DT.float32, DT.bfloat16, DT.float32r

    nc = bacc.Bacc("TRN2", target_bir_lowering=False, debug=False, num_devices=n_cores)

    x_d = nc.dram_tensor("x", [s_pc, H], f32, kind="ExternalInput").ap()
    g_d = nc.dram_tensor("lng", [1, H], f32, kind="ExternalInput").ap()
    b_d = nc.dram_tensor("lnb", [1, H], f32, kind="ExternalInput").ap()
    cos_d = nc.dram_tensor("cosT", [D, s_pc], f32, kind="ExternalInput").ap()
    sin_d = nc.dram_tensor("sinTs", [D, s_pc], f32, kind="ExternalInput").ap()
    wq_d = nc.dram_tensor("wqt", [128, QH, HB, 128], bf16, kind="ExternalInput").ap()
    wk_d = nc.dram_tensor("wkt", [128, KVH, HB, 128], bf16, kind="ExternalInput").ap()
    wv_d = nc.dram_tensor("wvt", [128, HB, KVH * D], bf16, kind="ExternalInput").ap()
    wo_d = nc.dram_tensor("wot", [128, HB, HB, 128], bf16, kind="ExternalInput").ap()
    sc_d = nc.dram_tensor("wscal", [1, 4], f32, kind="ExternalInput").ap()
    onesr_d = nc.dram_tensor("onesr", [128, 1], f32r, kind="ExternalInput").ap()
    y_d = nc.dram_tensor("yT", [H, s_pc], f32, kind="ExternalOutput").ap()

    with tile.TileContext(nc) as tc:
        _body(nc, tc, n_cores, s_pc, NT, KT,
              x_d, g_d, b_d, cos_d, sin_d, wq_d, wk_d, wv_d, wo_d, sc_d,
              onesr_d, y_d)
    nc.compile()
    return nc


def _body(nc, tc, n_cores, s_pc, NT, KT,
          x_d, g_d, b_d, cos_d, sin_d, wq_d, wk_d, wv_d, wo_d, sc_d,
          onesr_d, y_d):
    f32, bf16, f32r = DT.float32, DT.bfloat16, DT.float32r
    sync, vec, act, pe, gp = nc.sync, nc.vector, nc.scalar, nc.tensor, nc.gpsimd

    from contextlib import ExitStack

    def bcast_row(psb_pool, ones1, row, out_sb, n, name):
        """Replicate [1, n] row across 128 partitions via K=1 fp32 matmul
        (exact: 1.0 * a) then copy PSUM->SBUF."""
        for i, n0 in enumerate(range(0, n, 512)):
            nn = min(512, n - n0)
            ps = psb_pool.tile([128, 512], f32, name=f"{name}_ps{i}", tag="psb")
            pe.matmul(ps[:, 0:nn], ones1, row[:, n0:n0 + nn],
                      start=True, stop=True)
            vec.tensor_copy(out_sb[:, n0:n0 + nn], ps[:, 0:nn])

    es = ExitStack()
    with es:
        # ---------------- long-lived pools ----------------
        constp = es.enter_context(tc.tile_pool(name="constp", bufs=1))
        dramp = es.enter_context(tc.tile_pool(name="dramp", bufs=1, space="DRAM"))
        xTp = es.enter_context(tc.tile_pool(name="xTp", bufs=1))
        qTp = es.enter_context(tc.tile_pool(name="qTp", bufs=1))
        aop = es.enter_context(tc.tile_pool(name="aop", bufs=1))

        cosS = constp.tile([D, s_pc], f32, name="cosS", tag="cosS")
        sinS = constp.tile([D, s_pc], f32, name="sinS", tag="sinS")
        sync.dma_start(cosS, cos_d)
        sync.dma_start(sinS, sin_d)
        ones1 = constp.tile([1, 128], f32, name="ones1", tag="ones1")
        vec.memset(ones1, 1.0)
        scal_sb = constp.tile([128, 4], f32, name="scal_sb", tag="scal_sb")
        scal_row = constp.tile([1, 4], f32, name="scal_row", tag="scal_row")
        sync.dma_start(scal_row, sc_d)
        sw_q, sw_k, sw_v, sw_o = (scal_sb[:, i:i + 1] for i in range(4))
        ones_sb = constp.tile([128, 1], f32r, name="ones_sb", tag="ones_sb")
        sync.dma_start(ones_sb, onesr_d)

        # quantized+transposed activations [hid, tok] as bf16 integers
        xT = xTp.tile([128, HB, s_pc], bf16, name="xT", tag="xT")
        # per-token dequant scale r_i = clip(absmax,1e-5)/127, replicated on all partitions
        R = xTp.tile([128, s_pc], f32, name="R", tag="R")
        r_dram = dramp.tile([1, s_pc], f32, name="r_dram", tag="r_dram")

        qTall = qTp.tile([128, QH, s_pc], f32r, name="qTall", tag="qTall")
        aoall = aop.tile([128, QH, s_pc], f32, name="aoall", tag="aoall")
        acc = aop.tile([128, s_pc], f32, name="acc", tag="acc")
        vec.memset(acc, 0.0)

        # collective buffers
        ksrc = dramp.tile([KVH, D, s_pc], f32r, name="ksrc", tag="ksrc")
        vsrc = dramp.tile([NT, 128, KVH * D], f32r, name="vsrc", tag="vsrc")
        KG = dramp.tile([n_cores, KVH, D, s_pc], f32r, name="KG", tag="KG",
                        addr_space="Shared")
        VG = dramp.tile([n_cores, NT, 128, KVH * D], f32r, name="VG", tag="VG",
                        addr_space="Shared")

        r_tiles = []

        # per-token scale tiles (partition layout) -- live into phase 2
        for t in range(NT):
            r_t = constp.tile([128, 1], f32, name=f"r_{t}", tag=f"r_{t}")
            r_tiles.append(r_t)

        # ---------------- phase 1: layernorm + act quant ----------------
        with tc.tile_pool(name="lnp", bufs=1) as lnp, \
             tc.tile_pool(name="gbp", bufs=1) as gbp, \
             tc.tile_pool(name="statp", bufs=1) as statp, \
             tc.tile_pool(name="psb1", bufs=2, space="PSUM") as psb1, \
             tc.tile_pool(name="xqp", bufs=2) as xqp:
            Gt = gbp.tile([128, H], f32, name="Gt", tag="Gt")
            Bt = gbp.tile([128, H], f32, name="Bt", tag="Bt")
            grow = gbp.tile([1, H], f32, name="grow", tag="grow")
            brow = gbp.tile([1, H], f32, name="brow", tag="brow")
            sync.dma_start(grow, g_d)
            sync.dma_start(brow, b_d)
            bcast_row(psb1, ones1, grow, Gt, H, "g")
            bcast_row(psb1, ones1, brow, Bt, H, "b")
            bcast_row(psb1, ones1, scal_row, scal_sb, 4, "sc")

            for t in range(NT):
                xs = lnp.tile([128, H], f32, name=f"xs{t}", tag="xs")
                scr = lnp.tile([128, H], f32, name=f"scr{t}", tag="scr")
                sync.dma_start(xs, x_d[t * 128:(t + 1) * 128, :])

                nsum = statp.tile([128, 1], f32, name=f"nsum{t}", tag=f"nsum{t}")
                vec.tensor_reduce(nsum, xs, axis=AX.X, op=ALU.add, negate=True)
                nmu = statp.tile([128, 1], f32, name=f"nmu{t}", tag=f"nmu{t}")
                vec.tensor_scalar_mul(nmu, nsum, 1.0 / H)
                sumsq = statp.tile([128, 1], f32, name=f"sumsq{t}", tag=f"sumsq{t}")
                act.activation(scr, xs, AF.Square, bias=nmu, scale=1.0,
                               accum_out=sumsq)
                varv = statp.tile([128, 1], f32, name=f"varv{t}", tag=f"varv{t}")
                vec.tensor_scalar(varv, sumsq, 1.0 / H, LN_EPS, ALU.mult, ALU.add)
                stdv = statp.tile([128, 1], f32, name=f"stdv{t}", tag=f"stdv{t}")
                act.activation(stdv, varv, AF.Sqrt)
                rstd = statp.tile([128, 1], f32, name=f"rstd{t}", tag=f"rstd{t}")
                vec.reciprocal(rstd, stdv)
                nmr = statp.tile([128, 1], f32, name=f"nmr{t}", tag=f"nmr{t}")
                vec.tensor_mul(nmr, nmu, rstd)
                # normed = x*rstd + (-mu*rstd), then *g + b (in place)
                act.activation(xs, xs, AF.Identity, bias=nmr, scale=rstd)
                vec.tensor_mul(xs, xs, Gt)
                vec.tensor_add(xs, xs, Bt)

                am = statp.tile([128, 1], f32, name=f"am{t}", tag=f"am{t}")
                vec.tensor_reduce(am, xs, axis=AX.X, op=ALU.max,
                                  apply_absolute_value=True)
                amc = statp.tile([128, 1], f32, name=f"amc{t}", tag=f"amc{t}")
                vec.tensor_scalar_max(amc, am, 1e-5)
                r_t = r_tiles[t]
                vec.tensor_scalar_mul(r_t, amc, 1.0 / QB)
                inv = statp.tile([128, 1], f32, name=f"inv{t}", tag=f"inv{t}")
                vec.reciprocal(inv, amc)
                scq = statp.tile([128, 1], f32, name=f"scq{t}", tag=f"scq{t}")
                vec.tensor_scalar_mul(scq, inv, QB)

                # n = round(normed * scq), exact via +C trick; write as bf16 ints
                vec.tensor_scalar(scr, xs, scq, ROUND_C, ALU.mult, ALU.add)
                xq = xqp.tile([128, H], bf16, name=f"xq{t}", tag="xq")
                vec.tensor_scalar_add(xq, scr, -ROUND_C)

                # transpose into [hid, tok] layout (DMA xbar transpose, bf16)
                for h in range(HB):
                    sync.dma_start(xT[:, h, t * 128:(t + 1) * 128],
                                   xq[:, h * 128:(h + 1) * 128], transpose=True)
                # export per-token scale
                sync.dma_start(r_dram[0, t * 128:(t + 1) * 128], r_t[:, 0])

            r_row = constp.tile([1, s_pc], f32, name="r_row", tag="r_row")
            sync.dma_start(r_row, r_dram[:])
            bcast_row(psb1, ones1, r_row, R, s_pc, "r")

        # ---------------- phase 2: K,V projections + rope + gather ----------------
        with tc.tile_pool(name="wkvp", bufs=3) as wkvp, \
             tc.tile_pool(name="pskv", bufs=2, space="PSUM") as pskv, \
             tc.tile_pool(name="psv", bufs=1, space="PSUM") as psvp, \
             tc.tile_pool(name="kdrp", bufs=2) as kdrp:
            # K projection: kT[feat, tok] per kv head
            for f in range(KVH):
                wk_sb = wkvp.tile([128, HB, 128], bf16, name=f"wk{f}", tag="wkv")
                sync.dma_start(wk_sb, wk_d[:, f, :, :])
                ps = pskv.tile([128, s_pc], f32, name=f"psk{f}", tag="pskv")
                for k in range(HB):
                    pe.matmul(ps, wk_sb[:, k, :], xT[:, k, :],
                              start=(k == 0), stop=(k == HB - 1))
                kdr = kdrp.tile([128, s_pc], f32, name=f"kdr{f}", tag="kdr")
                vec.scalar_tensor_tensor(kdr, ps, sw_k, R, op0=ALU.mult,
                                         op1=ALU.mult)
                # rope
                rot = kdrp.tile([128, s_pc], f32, name=f"krot{f}", tag="krot")
                sync.dma_start(rot[0:64, :], kdr[64:128, :])
                sync.dma_start(rot[64:128, :], kdr[0:64, :])
                vec.tensor_mul(rot, rot, sinS)
                kcos = kdrp.tile([128, s_pc], f32, name=f"kcos{f}", tag="kcos")
                vec.tensor_mul(kcos, kdr, cosS)
                krp = kdrp.tile([128, s_pc], f32r, name=f"krp{f}", tag="krp")
                vec.tensor_add(krp, kcos, rot)
                sync.dma_start(ksrc[f], krp)

            # V projection: v[tok, feat] (x^T as stationary, weights moving)
            psvs = [psvp.tile([128, KVH * D], f32, name=f"psv{t}",
                              tag=f"psv{t}") for t in range(NT)]
            for k in range(HB):
                wv_sb = wkvp.tile([128, KVH * D], bf16, name=f"wv{k}",
                                  tag="wv")
                sync.dma_start(wv_sb, wv_d[:, k, :])
                for t in range(NT):
                    for n0 in (0, 512):
                        pe.matmul(psvs[t][:, n0:n0 + 512],
                                  xT[:, k, t * 128:(t + 1) * 128],
                                  wv_sb[:, n0:n0 + 512],
                                  start=(k == 0), stop=(k == HB - 1))
            for t in range(NT):
                vdr = kdrp.tile([128, KVH * D], f32r, name=f"vdr{t}", tag="vdr")
                vec.tensor_scalar(vdr, psvs[t], r_tiles[t], sw_v,
                                  ALU.mult, ALU.mult)
                sync.dma_start(vsrc[t], vdr)

            gp.collective_compute(
                "AllGather", ALU.bypass,
                replica_groups=[list(range(n_cores))],
                ins=[ksrc.opt()], outs=[KG.opt()])
            gp.collective_compute(
                "AllGather", ALU.bypass,
                replica_groups=[list(range(n_cores))],
                ins=[vsrc.opt()], outs=[VG.opt()])

        # ---------------- phase 3: Q projection + rope ----------------
        with tc.tile_pool(name="wqp", bufs=3) as wqp, \
             tc.tile_pool(name="psq", bufs=4, space="PSUM") as psqp, \
             tc.tile_pool(name="qdrp", bufs=2) as qdrp:
            for f in range(QH):
                wq_sb = wqp.tile([128, HB, 128], bf16, name=f"wq{f}", tag="wq")
                sync.dma_start(wq_sb, wq_d[:, f, :, :])
                ps = psqp.tile([128, s_pc], f32, name=f"psq{f}", tag="psq")
                for k in range(HB):
                    pe.matmul(ps, wq_sb[:, k, :], xT[:, k, :],
                              start=(k == 0), stop=(k == HB - 1))
                qdr = qdrp.tile([128, s_pc], f32, name=f"qdr{f}", tag="qdr")
                vec.scalar_tensor_tensor(qdr, ps, sw_q, R, op0=ALU.mult,
                                         op1=ALU.mult)
                rot = qdrp.tile([128, s_pc], f32, name=f"qrot{f}", tag="qrot")
                sync.dma_start(rot[0:64, :], qdr[64:128, :])
                sync.dma_start(rot[64:128, :], qdr[0:64, :])
                vec.tensor_mul(rot, rot, sinS)
                qcos = qdrp.tile([128, s_pc], f32, name=f"qcos{f}", tag="qcos")
                vec.tensor_mul(qcos, qdr, cosS)
                vec.tensor_add(qTall[:, f, :], qcos, rot)

        # ---------------- phase 4: attention ----------------
        with tc.tile_pool(name="kgp", bufs=2) as kgp, \
             tc.tile_pool(name="vgp", bufs=2) as vgp, \
             tc.tile_pool(name="ep", bufs=2) as ep, \
             tc.tile_pool(name="pss", bufs=3, space="PSUM") as pssp, \
             tc.tile_pool(name="psd", bufs=2, space="PSUM") as psdp, \
             tc.tile_pool(name="pso", bufs=2, space="PSUM") as psop, \
             tc.tile_pool(name="psb4", bufs=1, space="PSUM") as psb4, \
             tc.tile_pool(name="dnp", bufs=2) as dnp:
            for g in range(KVH):
                kg_sb = kgp.tile([128, n_cores, s_pc], f32r, name=f"kg{g}",
                                 tag="kg")
                sync.dma_start(kg_sb, KG[:, g, :, :].rearrange("r d t -> d r t"))
                vg_sb = vgp.tile([128, KT, 128], f32r, name=f"vg{g}", tag="vg")
                sync.dma_start(
                    vg_sb,
                    VG.rearrange("r t p f -> p (r t) f")[:, :,
                                                         g * 128:(g + 1) * 128])
                for j in range(NREP):
                    hq = g * NREP + j
                    qh_ap = qTall[:, hq, :]
                    eT = ep.tile([128, KT, s_pc], f32r, name=f"eT{g}_{j}",
                                 tag="eT")
                    for kt in range(KT):
                        pss = pssp.tile([128, s_pc], f32,
                                        name=f"pss{g}_{j}_{kt}", tag="pss")
                        pe.matmul(pss,
                                  kg_sb[:, kt // NT,
                                        (kt % NT) * 128:(kt % NT) * 128 + 128]
                                  ,
                                  qh_ap, start=True, stop=True)
                        act.activation(eT[:, kt, :], pss, AF.Exp,
                                       scale=SM_SCALE)
                    # denominators: ones^T @ eT accumulated over key tiles
                    psd = psdp.tile([1, s_pc], f32, name=f"psd{g}_{j}",
                                    tag="psd")
                    for kt in range(KT):
                        pe.matmul(psd, ones_sb,
                                  eT[:, kt, :],
                                  start=(kt == 0), stop=(kt == KT - 1))
                    # attention output (unnormalized): v^T as stationary
                    pso = psop.tile([128, s_pc], f32, name=f"pso{g}_{j}",
                                    tag="pso")
                    for kt in range(KT):
                        pe.matmul(pso, vg_sb[:, kt, :],
                                  eT[:, kt, :],
                                  start=(kt == 0), stop=(kt == KT - 1))
                    # normalize by softmax denominator
                    dnrow = dnp.tile([1, s_pc], f32, name=f"dnrow{g}_{j}",
                                     tag="dnrow")
                    act.activation(dnrow, psd, AF.Copy)
                    dps = psb4.tile([128, s_pc], f32, name=f"dps{g}_{j}",
                                    tag="dps")
                    pe.matmul(dps, ones1, dnrow, start=True, stop=True)
                    dnr = dnp.tile([128, s_pc], f32, name=f"dnr{g}_{j}",
                                   tag="dnr")
                    vec.reciprocal(dnr, dps)
                    vec.tensor_tensor(aoall[:, hq, :], pso, dnr, ALU.mult)
                    vec.tensor_tensor(acc, acc, aoall[:, hq, :], ALU.max)
                    vec.scalar_tensor_tensor(acc, aoall[:, hq, :], -1.0, acc,
                                             op0=ALU.mult, op1=ALU.max)

        # ---------------- phase 5: re-quant + O projection ----------------
        with tc.tile_pool(name="q2p", bufs=1) as q2p, \
             tc.tile_pool(name="t1p", bufs=2) as t1p, \
             tc.tile_pool(name="wop", bufs=3) as wop, \
             tc.tile_pool(name="psy", bufs=4, space="PSUM") as psyp, \
             tc.tile_pool(name="psb5", bufs=1, space="PSUM") as psb5, \
             tc.tile_pool(name="yp", bufs=2) as yp:
            # per-token absmax over the partition dim: halving tree
            # (acc is already elementwise |.|-accumulated, all >= 0)
            tmp = q2p.tile([64, s_pc], f32, name="tmphalf", tag="tmphalf")
            cur = 128
            while cur > 1:
                h = cur // 2
                sync.dma_start(tmp[0:h, :], acc[h:cur, :])
                vec.tensor_tensor(acc[0:h, :], acc[0:h, :], tmp[0:h, :],
                                  ALU.max)
                cur = h
            r2row = q2p.tile([1, s_pc], f32, name="r2row", tag="r2row")
            vec.tensor_scalar(r2row, acc[0:1, :], 1e-5, 1.0 / QB,
                              ALU.max, ALU.mult)
            R2 = q2p.tile([128, s_pc], f32, name="R2", tag="R2")
            bcast_row(psb5, ones1, r2row, R2, s_pc, "r2")
            sc2 = q2p.tile([128, s_pc], f32, name="sc2", tag="sc2")
            vec.reciprocal(sc2, R2)
            ao2 = q2p.tile([128, HB, s_pc], bf16, name="ao2", tag="ao2")
            for f in range(QH):
                t1 = t1p.tile([128, s_pc], f32, name=f"t1_{f}", tag="t1")
                vec.tensor_tensor(t1, aoall[:, f, :], sc2, ALU.mult)
                vec.tensor_scalar(ao2[:, f, :], t1, ROUND_C, -ROUND_C,
                                  ALU.add, ALU.add)

            for f in range(HB):
                wo_sb = wop.tile([128, HB, 128], bf16, name=f"wo{f}", tag="wo")
                sync.dma_start(wo_sb, wo_d[:, f, :, :])
                ps = psyp.tile([128, s_pc], f32, name=f"psy{f}", tag="psy")
                for k in range(HB):
                    pe.matmul(ps, wo_sb[:, k, :], ao2[:, k, :],
                              start=(k == 0), stop=(k == HB - 1))
                yT_sb = yp.tile([128, s_pc], f32, name=f"yT{f}", tag="yT")
                vec.scalar_tensor_tensor(yT_sb, ps, sw_o, R2, op0=ALU.mult,
                                         op1=ALU.mult)
                sync.dma_start(y_d[f * 128:(f + 1) * 128, :], yT_sb)


# ---------------------------------------------------------------------------
# host side
# ---------------------------------------------------------------------------

def _weight_quant_host(W):
    """Mimic reference _weight_quant: returns ternary m in {-1,0,1} and the
    effective dequant scale (1/scale) as fp32."""
    W = np.asarray(W, dtype=np.float32)
    mean_abs = np.float32(np.mean(np.abs(W), dtype=np.float64))
    clipped = np.maximum(mean_abs, np.float32(1e-5))
    scale = np.float32(1.0) / clipped
    m = np.clip(np.round(W * scale), -1.0, 1.0).astype(np.float32)
    sw = np.float32(1.0) / scale    # dequant scale applied after int matmul
    return m, float(sw)


def _prep_weights(Wq, Wk, Wv, Wo):
    mq, swq = _weight_quant_host(Wq)
    mk, swk = _weight_quant_host(Wk)
    mv, swv = _weight_quant_host(Wv)
    mo, swo = _weight_quant_host(Wo)

    def blocked(mT, fb):  # mT: [H, out] -> [128, fb, HB, 128]
        return np.ascontiguousarray(
            mT.reshape(HB, 128, fb, 128).transpose(1, 2, 0, 3)
        ).astype(np.float32)

    wqt = blocked(mq.T, QH)
    wkt = blocked(mk.T, KVH)
    wot = blocked(mo.T, HB)
    wvt = np.ascontiguousarray(
        mv.T.reshape(HB, 128, KVH * D).transpose(1, 0, 2)).astype(np.float32)
    import ml_dtypes
    tob = lambda a: a.astype(ml_dtypes.bfloat16)
    return (tob(wqt), tob(wkt), tob(wvt), tob(wot),
            np.array([[swq, swk, swv, swo]], dtype=np.float32))


def _rope_tables(S):
    inv = (1.0 / (10000.0 ** (np.arange(0, D, 2, dtype=np.float32)
                              / np.float32(D)))).astype(np.float32)
    pos = np.arange(S, dtype=np.float32)
    fr = pos[:, None] * inv[None, :]          # [S, 64]
    emb = np.concatenate([fr, fr], axis=1)    # [S, D]
    cosT = np.cos(emb).T.astype(np.float32).copy()   # [D, S]
    sinT = np.sin(emb).T.astype(np.float32).copy()
    sinT[0:64, :] *= -1.0                      # sign baked for rotate-half
    return cosT, sinT


def _in_maps(inputs, n_cores=8, s_pc=256):
    hs = np.asarray(inputs["hidden_states"], dtype=np.float32)
    ln_w = np.asarray(inputs["ln_w"], dtype=np.float32).reshape(1, H)
    ln_b = np.asarray(inputs["ln_b"], dtype=np.float32).reshape(1, H)
    wqt, wkt, wvt, wot, wscal = _prep_weights(
        inputs["Wq"], inputs["Wk"], inputs["Wv"], inputs["Wo"])
    S = hs.shape[1]
    cosT, sinT = _rope_tables(S)
    maps = []
    for c in range(n_cores):
        sl = slice(c * s_pc, (c + 1) * s_pc)
        maps.append({
            "x": np.ascontiguousarray(hs[0, sl, :]),
            "lng": ln_w, "lnb": ln_b,
            "cosT": np.ascontiguousarray(cosT[:, sl]),
            "sinTs": np.ascontiguousarray(sinT[:, sl]),
            "wqt": wqt, "wkt": wkt, "wvt": wvt, "wot": wot,
            "wscal": wscal,
            "onesr": np.ones((128, 1), dtype=np.float32),
        })
    return maps


_CACHED = {}


def _run(inputs, trace=False, n_cores=8, s_pc=256):
    from concourse.bass_utils import run_bass_kernel_spmd
    key = (n_cores, s_pc)
    if key not in _CACHED:
        _CACHED[key] = build(n_cores, s_pc)
    nc = _CACHED[key]
    maps = _in_maps(inputs, n_cores, s_pc)
    res = run_bass_kernel_spmd(nc, maps, list(range(n_cores)), trace=trace)
    parts = [res.results[c]["yT"].T for c in range(n_cores)]
    y = np.concatenate(parts, axis=0)[None, :, :].astype(np.float32)
    return y, res.exec_time_ns


def kernel(**inputs):
    y, _ = _run(inputs, trace=False)
    return y


# revision 18
# speedup vs baseline: 8.6053x; 8.6053x over previous
# BitNet GQA attention block on 8 Trainium2 NeuronCores.
#
# Sharding: data parallel over sequence (256 tokens/core). K/V are computed
# per-core for the local tokens, RoPE'd, then AllGathered so every core can
# run full (non-causal) attention for its own query tokens. Projections run
# as integer-exact bf16 matmuls (8-bit quantized activations are integers
# <=127, ternary weights are -1/0/1 -- both exact in bf16; PSUM accumulates
# in fp32 and |dot| < 2^24 so results are exact). Attention matmuls use
# float32r (full PE rate at free-dim >= 256).
import math

import numpy as np

import concourse.bacc as bacc
import concourse.bass as bass
import concourse.bass_isa as bass_isa
import concourse.mybir as mybir
import concourse.tile as tile

DT = mybir.dt
AF = mybir.ActivationFunctionType
ALU = mybir.AluOpType
AX = mybir.AxisListType

H = 4096
QH, KVH, D = 32, 8, 128     # query heads, kv heads, head dim
HB = H // 128               # 32 hidden blocks
NREP = QH // KVH
ROUND_C = 12582912.0        # 1.5 * 2**23: fp32 add forces round-to-nearest-even int
LN_EPS = 1e-5
QB = 127.0
SM_SCALE = 1.0 / math.sqrt(128.0)


def build(n_cores=8, s_pc=256, stub_collectives=False, body_reps=1):
    """Build the SPMD Bass program (identical on all cores; per-core data via inputs)."""
    NT = s_pc // 128            # token tiles per core
    S = s_pc * n_cores
    KT = S // 128               # key-token tiles after gather
    f32, bf16, f32r = DT.float32, DT.bfloat16, DT.float32r

    nc = bacc.Bacc("TRN2", target_bir_lowering=False, debug=False, num_devices=n_cores)

    x_d = nc.dram_tensor("x", [s_pc, H], f32, kind="ExternalInput").ap()
    g_d = nc.dram_tensor("lng", [1, H], f32, kind="ExternalInput").ap()
    b_d = nc.dram_tensor("lnb", [1, H], f32, kind="ExternalInput").ap()
    cos_d = nc.dram_tensor("cosT", [D, s_pc], f32, kind="ExternalInput").ap()
    sin_d = nc.dram_tensor("sinTs", [D, s_pc], f32, kind="ExternalInput").ap()
    wq_d = nc.dram_tensor("wqt", [128, QH, HB, 128], bf16, kind="ExternalInput").ap()
    wk_d = nc.dram_tensor("wkt", [128, KVH, HB, 128], bf16, kind="ExternalInput").ap()
    wv_d = nc.dram_tensor("wvt", [128, HB, KVH * D], bf16, kind="ExternalInput").ap()
    wo_d = nc.dram_tensor("wot", [128, HB, HB, 128], bf16, kind="ExternalInput").ap()
    sc_d = nc.dram_tensor("wscal", [1, 4], f32, kind="ExternalInput").ap()
    onesr_d = nc.dram_tensor("onesr", [128, 1], f32r, kind="ExternalInput").ap()
    y_d = nc.dram_tensor("yT", [H, s_pc], f32, kind="ExternalOutput").ap()

    with tile.TileContext(nc) as tc:
        for rep in range(body_reps):
            _body(nc, tc, n_cores, s_pc, NT, KT,
                  x_d, g_d, b_d, cos_d, sin_d, wq_d, wk_d, wv_d, wo_d, sc_d,
                  onesr_d, y_d, stub_collectives, pfx=f"r{rep}_")
    nc.compile()
    return nc


def _body(nc, tc, n_cores, s_pc, NT, KT,
          x_d, g_d, b_d, cos_d, sin_d, wq_d, wk_d, wv_d, wo_d, sc_d,
          onesr_d, y_d, stub_collectives=False, pfx=""):
    f32, bf16, f32r = DT.float32, DT.bfloat16, DT.float32r
    sync, vec, act, pe, gp = nc.sync, nc.vector, nc.scalar, nc.tensor, nc.gpsimd

    from contextlib import ExitStack

    def bcast_row(psb_pool, ones1, row, out_sb, n, name):
        """Replicate [1, n] row across 128 partitions via K=1 fp32 matmul
        (exact: 1.0 * a) then copy PSUM->SBUF."""
        for i, n0 in enumerate(range(0, n, 512)):
            nn = min(512, n - n0)
            ps = psb_pool.tile([128, 512], f32, name=f"{name}_ps{i}", tag="psb")
            pe.matmul(ps[:, 0:nn], ones1, row[:, n0:n0 + nn],
                      start=True, stop=True)
            vec.tensor_copy(out_sb[:, n0:n0 + nn], ps[:, 0:nn])

    es = ExitStack()
    with es:
        # ---------------- long-lived pools ----------------
        constp = es.enter_context(tc.tile_pool(name=pfx + "constp", bufs=1))
        dramp = es.enter_context(tc.tile_pool(name=pfx + "dramp", bufs=1, space="DRAM"))
        xTp = es.enter_context(tc.tile_pool(name=pfx + "xTp", bufs=1))
        qTp = es.enter_context(tc.tile_pool(name=pfx + "qTp", bufs=1))
        aop = es.enter_context(tc.tile_pool(name=pfx + "aop", bufs=1))

        cosS = constp.tile([D, s_pc], f32, name="cosS", tag="cosS")
        sinS = constp.tile([D, s_pc], f32, name="sinS", tag="sinS")
        sync.dma_start(cosS, cos_d)
        sync.dma_start(sinS, sin_d)
        ones1 = constp.tile([1, 128], f32, name="ones1", tag="ones1")
        vec.memset(ones1, 1.0)
        scal_sb = constp.tile([128, 4], f32, name="scal_sb", tag="scal_sb")
        scal_row = constp.tile([1, 4], f32, name="scal_row", tag="scal_row")
        sync.dma_start(scal_row, sc_d)
        sw_q, sw_k, sw_v, sw_o = (scal_sb[:, i:i + 1] for i in range(4))
        ones_sb = constp.tile([128, 1], f32r, name="ones_sb", tag="ones_sb")
        sync.dma_start(ones_sb, onesr_d)

        # quantized+transposed activations [hid, tok] as bf16 integers
        xT = xTp.tile([128, HB, s_pc], bf16, name="xT", tag="xT")
        # per-token dequant scale r_i = clip(absmax,1e-5)/127, replicated on all partitions
        R = xTp.tile([128, s_pc], f32, name="R", tag="R")
        r_dram = dramp.tile([1, s_pc], f32, name="r_dram", tag="r_dram")

        qTall = qTp.tile([128, QH, s_pc], f32r, name="qTall", tag="qTall")
        aoall = aop.tile([128, QH, s_pc], f32, name="aoall", tag="aoall")
        acc = aop.tile([128, s_pc], f32, name="acc", tag="acc")
        vec.memset(acc, 0.0)

        # collective buffers
        ksrc = dramp.tile([KVH, D, s_pc], f32r, name="ksrc", tag="ksrc")
        vsrc = dramp.tile([NT, 128, KVH * D], f32r, name="vsrc", tag="vsrc")
        kv_space = "Local" if stub_collectives else "Shared"
        KG = dramp.tile([n_cores, KVH, D, s_pc], f32r, name="KG", tag="KG",
                        addr_space=kv_space)
        VG = dramp.tile([n_cores, NT, 128, KVH * D], f32r, name="VG", tag="VG",
                        addr_space=kv_space)

        r_tiles = []

        # per-token scale tiles (partition layout) -- live into phase 2
        for t in range(NT):
            r_t = constp.tile([128, 1], f32, name=f"r_{t}", tag=f"r_{t}")
            r_tiles.append(r_t)

        # ---------------- phase 1: layernorm + act quant ----------------
        with tc.tile_pool(name=pfx + "lnp", bufs=1) as lnp, \
             tc.tile_pool(name=pfx + "gbp", bufs=1) as gbp, \
             tc.tile_pool(name=pfx + "statp", bufs=1) as statp, \
             tc.tile_pool(name=pfx + "psb1", bufs=2, space="PSUM") as psb1, \
             tc.tile_pool(name=pfx + "xqp", bufs=2) as xqp:
            Gt = gbp.tile([128, H], f32, name="Gt", tag="Gt")
            Bt = gbp.tile([128, H], f32, name="Bt", tag="Bt")
            grow = gbp.tile([1, H], f32, name="grow", tag="grow")
            brow = gbp.tile([1, H], f32, name="brow", tag="brow")
            sync.dma_start(grow, g_d)
            sync.dma_start(brow, b_d)
            bcast_row(psb1, ones1, grow, Gt, H, "g")
            bcast_row(psb1, ones1, brow, Bt, H, "b")
            bcast_row(psb1, ones1, scal_row, scal_sb, 4, "sc")

            for t in range(NT):
                xs = lnp.tile([128, H], f32, name=f"xs{t}", tag="xs")
                scr = lnp.tile([128, H], f32, name=f"scr{t}", tag="scr")
                sync.dma_start(xs, x_d[t * 128:(t + 1) * 128, :])

                nsum = statp.tile([128, 1], f32, name=f"nsum{t}", tag=f"nsum{t}")
                vec.tensor_reduce(nsum, xs, axis=AX.X, op=ALU.add, negate=True)
                nmu = statp.tile([128, 1], f32, name=f"nmu{t}", tag=f"nmu{t}")
                vec.tensor_scalar_mul(nmu, nsum, 1.0 / H)
                sumsq = statp.tile([128, 1], f32, name=f"sumsq{t}", tag=f"sumsq{t}")
                act.activation(scr, xs, AF.Square, bias=nmu, scale=1.0,
                               accum_out=sumsq)
                varv = statp.tile([128, 1], f32, name=f"varv{t}", tag=f"varv{t}")
                vec.tensor_scalar(varv, sumsq, 1.0 / H, LN_EPS, ALU.mult, ALU.add)
                stdv = statp.tile([128, 1], f32, name=f"stdv{t}", tag=f"stdv{t}")
                act.activation(stdv, varv, AF.Sqrt)
                rstd = statp.tile([128, 1], f32, name=f"rstd{t}", tag=f"rstd{t}")
                vec.reciprocal(rstd, stdv)
                nmr = statp.tile([128, 1], f32, name=f"nmr{t}", tag=f"nmr{t}")
                vec.tensor_mul(nmr, nmu, rstd)
                # normed = x*rstd + (-mu*rstd), then *g + b (in place)
                act.activation(xs, xs, AF.Identity, bias=nmr, scale=rstd)
                vec.tensor_mul(xs, xs, Gt)
                vec.tensor_add(xs, xs, Bt)

                am = statp.tile([128, 1], f32, name=f"am{t}", tag=f"am{t}")
                vec.tensor_reduce(am, xs, axis=AX.X, op=ALU.max,
                                  apply_absolute_value=True)
                amc = statp.tile([128, 1], f32, name=f"amc{t}", tag=f"amc{t}")
                vec.tensor_scalar_max(amc, am, 1e-5)
                r_t = r_tiles[t]
                vec.tensor_scalar_mul(r_t, amc, 1.0 / QB)
                inv = statp.tile([128, 1], f32, name=f"inv{t}", tag=f"inv{t}")
                vec.reciprocal(inv, amc)
                scq = statp.tile([128, 1], f32, name=f"scq{t}", tag=f"scq{t}")
                vec.tensor_scalar_mul(scq, inv, QB)

                # n = round(normed * scq), exact via +C trick; write as bf16 ints
                vec.tensor_scalar(scr, xs, scq, ROUND_C, ALU.mult, ALU.add)
                xq = xqp.tile([128, H], bf16, name=f"xq{t}", tag="xq")
                vec.tensor_scalar_add(xq, scr, -ROUND_C)

                # transpose into [hid, tok] layout (DMA xbar transpose, bf16)
                for h in range(HB):
                    sync.dma_start(xT[:, h, t * 128:(t + 1) * 128],
                                   xq[:, h * 128:(h + 1) * 128], transpose=True)
                # export per-token scale
                sync.dma_start(r_dram[0, t * 128:(t + 1) * 128], r_t[:, 0])

            r_row = constp.tile([1, s_pc], f32, name="r_row", tag="r_row")
            sync.dma_start(r_row, r_dram[:])
            bcast_row(psb1, ones1, r_row, R, s_pc, "r")

        # ---------------- phase 2: K,V projections + rope + gather ----------------
        with tc.tile_pool(name=pfx + "wkvp", bufs=3) as wkvp, \
             tc.tile_pool(name=pfx + "pskv", bufs=2, space="PSUM") as pskv, \
             tc.tile_pool(name=pfx + "psv", bufs=1, space="PSUM") as psvp, \
             tc.tile_pool(name=pfx + "kdrp", bufs=2) as kdrp:
            # K projection: kT[feat, tok] per kv head
            for f in range(KVH):
                wk_sb = wkvp.tile([128, HB, 128], bf16, name=f"wk{f}", tag="wkv")
                sync.dma_start(wk_sb, wk_d[:, f, :, :])
                ps = pskv.tile([128, s_pc], f32, name=f"psk{f}", tag="pskv")
                for k in range(HB):
                    pe.matmul(ps, wk_sb[:, k, :], xT[:, k, :],
                              start=(k == 0), stop=(k == HB - 1))
                kdr = kdrp.tile([128, s_pc], f32, name=f"kdr{f}", tag="kdr")
                vec.scalar_tensor_tensor(kdr, ps, sw_k, R, op0=ALU.mult,
                                         op1=ALU.mult)
                # rope
                rot = kdrp.tile([128, s_pc], f32, name=f"krot{f}", tag="krot")
                sync.dma_start(rot[0:64, :], kdr[64:128, :])
                sync.dma_start(rot[64:128, :], kdr[0:64, :])
                vec.tensor_mul(rot, rot, sinS)
                kcos = kdrp.tile([128, s_pc], f32, name=f"kcos{f}", tag="kcos")
                vec.tensor_mul(kcos, kdr, cosS)
                krp = kdrp.tile([128, s_pc], f32r, name=f"krp{f}", tag="krp")
                vec.tensor_add(krp, kcos, rot)
                sync.dma_start(ksrc[f], krp)

            # V projection: v[tok, feat] (x^T as stationary, weights moving)
            psvs = [psvp.tile([128, KVH * D], f32, name=f"psv{t}",
                              tag=f"psv{t}") for t in range(NT)]
            for k in range(HB):
                wv_sb = wkvp.tile([128, KVH * D], bf16, name=f"wv{k}",
                                  tag="wv")
                sync.dma_start(wv_sb, wv_d[:, k, :])
                for t in range(NT):
                    for n0 in (0, 512):
                        pe.matmul(psvs[t][:, n0:n0 + 512],
                                  xT[:, k, t * 128:(t + 1) * 128],
                                  wv_sb[:, n0:n0 + 512],
                                  start=(k == 0), stop=(k == HB - 1))
            for t in range(NT):
                vdr = kdrp.tile([128, KVH * D], f32r, name=f"vdr{t}", tag="vdr")
                vec.tensor_scalar(vdr, psvs[t], r_tiles[t], sw_v,
                                  ALU.mult, ALU.mult)
                sync.dma_start(vsrc[t], vdr)

            if stub_collectives:
                for r in range(n_cores):
                    sync.dma_start(KG[r], ksrc)
                    sync.dma_start(VG[r], vsrc)
            else:
                gp.collective_compute(
                    "AllGather", ALU.bypass,
                    replica_groups=[list(range(n_cores))],
                    ins=[ksrc.opt()], outs=[KG.opt()])
                gp.collective_compute(
                    "AllGather", ALU.bypass,
                    replica_groups=[list(range(n_cores))],
                    ins=[vsrc.opt()], outs=[VG.opt()])

        # ---------------- phase 3: Q projection + rope ----------------
        with tc.tile_pool(name=pfx + "wqp", bufs=3) as wqp, \
             tc.tile_pool(name=pfx + "psq", bufs=4, space="PSUM") as psqp, \
             tc.tile_pool(name=pfx + "qdrp", bufs=2) as qdrp:
            for f in range(QH):
                wq_sb = wqp.tile([128, HB, 128], bf16, name=f"wq{f}", tag="wq")
                sync.dma_start(wq_sb, wq_d[:, f, :, :])
                ps = psqp.tile([128, s_pc], f32, name=f"psq{f}", tag="psq")
                for k in range(HB):
                    pe.matmul(ps, wq_sb[:, k, :], xT[:, k, :],
                              start=(k == 0), stop=(k == HB - 1))
                qdr = qdrp.tile([128, s_pc], f32, name=f"qdr{f}", tag="qdr")
                vec.scalar_tensor_tensor(qdr, ps, sw_q, R, op0=ALU.mult,
                                         op1=ALU.mult)
                rot = qdrp.tile([128, s_pc], f32, name=f"qrot{f}", tag="qrot")
                sync.dma_start(rot[0:64, :], qdr[64:128, :])
                sync.dma_start(rot[64:128, :], qdr[0:64, :])
                vec.tensor_mul(rot, rot, sinS)
                qcos = qdrp.tile([128, s_pc], f32, name=f"qcos{f}", tag="qcos")
                vec.tensor_mul(qcos, qdr, cosS)
                vec.tensor_add(qTall[:, f, :], qcos, rot)

        # ---------------- phase 4: attention ----------------
        with tc.tile_pool(name=pfx + "kgp", bufs=2) as kgp, \
             tc.tile_pool(name=pfx + "vgp", bufs=2) as vgp, \
             tc.tile_pool(name=pfx + "ep", bufs=2) as ep, \
             tc.tile_pool(name=pfx + "pss", bufs=3, space="PSUM") as pssp, \
             tc.tile_pool(name=pfx + "psd", bufs=2, space="PSUM") as psdp, \
             tc.tile_pool(name=pfx + "pso", bufs=2, space="PSUM") as psop, \
             tc.tile_pool(name=pfx + "psb4", bufs=1, space="PSUM") as psb4, \
             tc.tile_pool(name=pfx + "dnp", bufs=2) as dnp:
            for g in range(KVH):
                kg_sb = kgp.tile([128, n_cores, s_pc], f32r, name=f"kg{g}",
                                 tag="kg")
                sync.dma_start(kg_sb, KG[:, g, :, :].rearrange("r d t -> d r t"))
                vg_sb = vgp.tile([128, KT, 128], f32r, name=f"vg{g}", tag="vg")
                sync.dma_start(
                    vg_sb,
                    VG.rearrange("r t p f -> p (r t) f")[:, :,
                                                         g * 128:(g + 1) * 128])
                for j in range(NREP):
                    hq = g * NREP + j
                    qh_ap = qTall[:, hq, :]
                    eT = ep.tile([128, KT, s_pc], f32r, name=f"eT{g}_{j}",
                                 tag="eT")
                    for kt in range(KT):
                        pss = pssp.tile([128, s_pc], f32,
                                        name=f"pss{g}_{j}_{kt}", tag="pss")
                        pe.matmul(pss,
                                  kg_sb[:, kt // NT,
                                        (kt % NT) * 128:(kt % NT) * 128 + 128]
                                  ,
                                  qh_ap, start=True, stop=True)
                        act.activation(eT[:, kt, :], pss, AF.Exp,
                                       scale=SM_SCALE)
                    # denominators: ones^T @ eT accumulated over key tiles
                    psd = psdp.tile([1, s_pc], f32, name=f"psd{g}_{j}",
                                    tag="psd")
                    for kt in range(KT):
                        pe.matmul(psd, ones_sb,
                                  eT[:, kt, :],
                                  start=(kt == 0), stop=(kt == KT - 1))
                    # attention output (unnormalized): v^T as stationary
                    pso = psop.tile([128, s_pc], f32, name=f"pso{g}_{j}",
                                    tag="pso")
                    for kt in range(KT):
                        pe.matmul(pso, vg_sb[:, kt, :],
                                  eT[:, kt, :],
                                  start=(kt == 0), stop=(kt == KT - 1))
                    # normalize by softmax denominator
                    dnrow = dnp.tile([1, s_pc], f32, name=f"dnrow{g}_{j}",
                                     tag="dnrow")
                    act.activation(dnrow, psd, AF.Copy)
                    dps = psb4.tile([128, s_pc], f32, name=f"dps{g}_{j}",
                                    tag="dps")
                    pe.matmul(dps, ones1, dnrow, start=True, stop=True)
                    dnr = dnp.tile([128, s_pc], f32, name=f"dnr{g}_{j}",
                                   tag="dnr")
                    vec.reciprocal(dnr, dps)
                    vec.tensor_tensor(aoall[:, hq, :], pso, dnr, ALU.mult)
                    vec.tensor_tensor(acc, acc, aoall[:, hq, :], ALU.max)
                    vec.scalar_tensor_tensor(acc, aoall[:, hq, :], -1.0, acc,
                                             op0=ALU.mult, op1=ALU.max)

        # ---------------- phase 5: re-quant + O projection ----------------
        with tc.tile_pool(name=pfx + "q2p", bufs=1) as q2p, \
             tc.tile_pool(name=pfx + "t1p", bufs=2) as t1p, \
             tc.tile_pool(name=pfx + "wop", bufs=3) as wop, \
             tc.tile_pool(name=pfx + "psy", bufs=4, space="PSUM") as psyp, \
             tc.tile_pool(name=pfx + "psb5", bufs=1, space="PSUM") as psb5, \
             tc.tile_pool(name=pfx + "yp", bufs=2) as yp:
            # per-token absmax over the partition dim: halving tree
            # (acc is already elementwise |.|-accumulated, all >= 0)
            tmp = q2p.tile([64, s_pc], f32, name="tmphalf", tag="tmphalf")
            cur = 128
            while cur > 1:
                h = cur // 2
                sync.dma_start(tmp[0:h, :], acc[h:cur, :])
                vec.tensor_tensor(acc[0:h, :], acc[0:h, :], tmp[0:h, :],
                                  ALU.max)
                cur = h
            r2row = q2p.tile([1, s_pc], f32, name="r2row", tag="r2row")
            vec.tensor_scalar(r2row, acc[0:1, :], 1e-5, 1.0 / QB,
                              ALU.max, ALU.mult)
            R2 = q2p.tile([128, s_pc], f32, name="R2", tag="R2")
            bcast_row(psb5, ones1, r2row, R2, s_pc, "r2")
            sc2 = q2p.tile([128, s_pc], f32, name="sc2", tag="sc2")
            vec.reciprocal(sc2, R2)
            ao2 = q2p.tile([128, HB, s_pc], bf16, name="ao2", tag="ao2")
            for f in range(QH):
                t1 = t1p.tile([128, s_pc], f32, name=f"t1_{f}", tag="t1")
                vec.tensor_tensor(t1, aoall[:, f, :], sc2, ALU.mult)
                vec.tensor_scalar(ao2[:, f, :], t1, ROUND_C, -ROUND_C,
                                  ALU.add, ALU.add)

            for f in range(HB):
                wo_sb = wop.tile([128, HB, 128], bf16, name=f"wo{f}", tag="wo")
                sync.dma_start(wo_sb, wo_d[:, f, :, :])
                ps = psyp.tile([128, s_pc], f32, name=f"psy{f}", tag="psy")
                for k in range(HB):
                    pe.matmul(ps, wo_sb[:, k, :], ao2[:, k, :],
                              start=(k == 0), stop=(k == HB - 1))
                yT_sb = yp.tile([128, s_pc], f32, name=f"yT{f}", tag="yT")
                vec.scalar_tensor_tensor(yT_sb, ps, sw_o, R2, op0=ALU.mult,
                                         op1=ALU.mult)
                sync.dma_start(y_d[f * 128:(f + 1) * 128, :], yT_sb)


# ---------------------------------------------------------------------------
# host side
# ---------------------------------------------------------------------------

def _weight_quant_host(W):
    """Mimic reference _weight_quant: returns ternary m in {-1,0,1} and the
    effective dequant scale (1/scale) as fp32."""
    W = np.asarray(W, dtype=np.float32)
    mean_abs = np.float32(np.mean(np.abs(W), dtype=np.float64))
    clipped = np.maximum(mean_abs, np.float32(1e-5))
    scale = np.float32(1.0) / clipped
    m = np.clip(np.round(W * scale), -1.0, 1.0).astype(np.float32)
    sw = np.float32(1.0) / scale    # dequant scale applied after int matmul
    return m, float(sw)


def _prep_weights(Wq, Wk, Wv, Wo):
    mq, swq = _weight_quant_host(Wq)
    mk, swk = _weight_quant_host(Wk)
    mv, swv = _weight_quant_host(Wv)
    mo, swo = _weight_quant_host(Wo)

    def blocked(mT, fb):  # mT: [H, out] -> [128, fb, HB, 128]
        return np.ascontiguousarray(
            mT.reshape(HB, 128, fb, 128).transpose(1, 2, 0, 3)
        ).astype(np.float32)

    wqt = blocked(mq.T, QH)
    wkt = blocked(mk.T, KVH)
    wot = blocked(mo.T, HB)
    wvt = np.ascontiguousarray(
        mv.T.reshape(HB, 128, KVH * D).transpose(1, 0, 2)).astype(np.float32)
    import ml_dtypes
    tob = lambda a: a.astype(ml_dtypes.bfloat16)
    return (tob(wqt), tob(wkt), tob(wvt), tob(wot),
            np.array([[swq, swk, swv, swo]], dtype=np.float32))


def _rope_tables(S):
    inv = (1.0 / (10000.0 ** (np.arange(0, D, 2, dtype=np.float32)
                              / np.float32(D)))).astype(np.float32)
    pos = np.arange(S, dtype=np.float32)
    fr = pos[:, None] * inv[None, :]          # [S, 64]
    emb = np.concatenate([fr, fr], axis=1)    # [S, D]
    cosT = np.cos(emb).T.astype(np.float32).copy()   # [D, S]
    sinT = np.sin(emb).T.astype(np.float32).copy()
    sinT[0:64, :] *= -1.0                      # sign baked for rotate-half
    return cosT, sinT


def _in_maps(inputs, n_cores=8, s_pc=256):
    hs = np.asarray(inputs["hidden_states"], dtype=np.float32)
    ln_w = np.asarray(inputs["ln_w"], dtype=np.float32).reshape(1, H)
    ln_b = np.asarray(inputs["ln_b"], dtype=np.float32).reshape(1, H)
    wqt, wkt, wvt, wot, wscal = _prep_weights(
        inputs["Wq"], inputs["Wk"], inputs["Wv"], inputs["Wo"])
    S = hs.shape[1]
    cosT, sinT = _rope_tables(S)
    maps = []
    for c in range(n_cores):
        sl = slice(c * s_pc, (c + 1) * s_pc)
        maps.append({
            "x": np.ascontiguousarray(hs[0, sl, :]),
            "lng": ln_w, "lnb": ln_b,
            "cosT": np.ascontiguousarray(cosT[:, sl]),
            "sinTs": np.ascontiguousarray(sinT[:, sl]),
            "wqt": wqt, "wkt": wkt, "wvt": wvt, "wot": wot,
            "wscal": wscal,
            "onesr": np.ones((128, 1), dtype=np.float32),
        })
    return maps


_CACHED = {}


def _run(inputs, trace=False, n_cores=8, s_pc=256):
    from concourse.bass_utils import run_bass_kernel_spmd
    key = (n_cores, s_pc)
    if key not in _CACHED:
        _CACHED[key] = build(n_cores, s_pc)
    nc = _CACHED[key]
    maps = _in_maps(inputs, n_cores, s_pc)
    res = run_bass_kernel_spmd(nc, maps, list(range(n_cores)), trace=trace)
    parts = [res.results[c]["yT"].T for c in range(n_cores)]
    y = np.concatenate(parts, axis=0)[None, :, :].astype(np.float32)
    return y, res.exec_time_ns


def kernel(**inputs):
    y, _ = _run(inputs, trace=False)
    return y


# revision 20
# speedup vs baseline: 10.7406x; 1.2481x over previous
# BitNet GQA attention block on 8 Trainium2 NeuronCores.
#
# Sharding: data parallel over sequence (256 tokens/core). K/V are computed
# per-core for the local tokens, RoPE'd, then AllGathered so every core can
# run full (non-causal) attention for its own query tokens. Projections run
# as integer-exact bf16 matmuls (8-bit quantized activations are integers
# <=127, ternary weights are -1/0/1 -- both exact in bf16; PSUM accumulates
# in fp32 and |dot| < 2^24 so results are exact). Attention matmuls use
# float32r (full PE rate at free-dim >= 256).
import math

import numpy as np

import concourse.bacc as bacc
import concourse.bass as bass
import concourse.bass_isa as bass_isa
import concourse.mybir as mybir
import concourse.tile as tile

DT = mybir.dt
AF = mybir.ActivationFunctionType
ALU = mybir.AluOpType
AX = mybir.AxisListType

H = 4096
QH, KVH, D = 32, 8, 128     # query heads, kv heads, head dim
HB = H // 128               # 32 hidden blocks
NREP = QH // KVH
ROUND_C = 12582912.0        # 1.5 * 2**23: fp32 add forces round-to-nearest-even int
LN_EPS = 1e-5
QB = 127.0
SM_SCALE = 1.0 / math.sqrt(128.0)


def build(n_cores=8, s_pc=256, stub_collectives=False, body_reps=1,
          skip_gb=False):
    """Build the SPMD Bass program (identical on all cores; per-core data via inputs)."""
    NT = s_pc // 128            # token tiles per core
    S = s_pc * n_cores
    KT = S // 128               # key-token tiles after gather
    f32, bf16, f32r = DT.float32, DT.bfloat16, DT.float32r

    nc = bacc.Bacc("TRN2", target_bir_lowering=False, debug=False, num_devices=n_cores)

    x_d = nc.dram_tensor("x", [s_pc, H], f32, kind="ExternalInput").ap()
    g_d = nc.dram_tensor("lng", [1, H], f32, kind="ExternalInput").ap()
    b_d = nc.dram_tensor("lnb", [1, H], f32, kind="ExternalInput").ap()
    cos_d = nc.dram_tensor("cosT", [D, s_pc], f32, kind="ExternalInput").ap()
    sin_d = nc.dram_tensor("sinTs", [D, s_pc], f32, kind="ExternalInput").ap()
    wq_d = nc.dram_tensor("wqt", [128, QH, HB, 128], bf16, kind="ExternalInput").ap()
    wk_d = nc.dram_tensor("wkt", [128, KVH, HB, 128], bf16, kind="ExternalInput").ap()
    wv_d = nc.dram_tensor("wvt", [128, HB, KVH * D], bf16, kind="ExternalInput").ap()
    wo_d = nc.dram_tensor("wot", [128, HB, HB, 128], bf16, kind="ExternalInput").ap()
    sc_d = nc.dram_tensor("wscal", [1, 4], f32, kind="ExternalInput").ap()
    onesr_d = nc.dram_tensor("onesr", [128, 1], f32r, kind="ExternalInput").ap()
    y_d = nc.dram_tensor("yT", [H, s_pc], f32, kind="ExternalOutput").ap()

    with tile.TileContext(nc) as tc:
        for rep in range(body_reps):
            _body(nc, tc, n_cores, s_pc, NT, KT,
                  x_d, g_d, b_d, cos_d, sin_d, wq_d, wk_d, wv_d, wo_d, sc_d,
                  onesr_d, y_d, stub_collectives, pfx=f"r{rep}_",
                  skip_gb=skip_gb)
    nc.compile()
    return nc


def _body(nc, tc, n_cores, s_pc, NT, KT,
          x_d, g_d, b_d, cos_d, sin_d, wq_d, wk_d, wv_d, wo_d, sc_d,
          onesr_d, y_d, stub_collectives=False, pfx="", skip_gb=False):
    f32, bf16, f32r = DT.float32, DT.bfloat16, DT.float32r
    sync, vec, act, pe, gp = nc.sync, nc.vector, nc.scalar, nc.tensor, nc.gpsimd

    from contextlib import ExitStack

    def bcast_row(psb_pool, ones1, row, out_sb, n, name):
        """Replicate [1, n] row across 128 partitions via K=1 fp32 matmul
        (exact: 1.0 * a) then copy PSUM->SBUF."""
        for i, n0 in enumerate(range(0, n, 512)):
            nn = min(512, n - n0)
            ps = psb_pool.tile([128, 512], f32, name=f"{name}_ps{i}", tag="psb")
            pe.matmul(ps[:, 0:nn], ones1, row[:, n0:n0 + nn],
                      start=True, stop=True)
            vec.tensor_copy(out_sb[:, n0:n0 + nn], ps[:, 0:nn])

    es = ExitStack()
    with es:
        # ---------------- long-lived pools ----------------
        constp = es.enter_context(tc.tile_pool(name=pfx + "constp", bufs=1))
        dramp = es.enter_context(tc.tile_pool(name=pfx + "dramp", bufs=1, space="DRAM"))
        xTp = es.enter_context(tc.tile_pool(name=pfx + "xTp", bufs=1))
        qTp = es.enter_context(tc.tile_pool(name=pfx + "qTp", bufs=1))
        aop = es.enter_context(tc.tile_pool(name=pfx + "aop", bufs=1))

        cosS = constp.tile([D, s_pc], f32, name="cosS", tag="cosS")
        sinS = constp.tile([D, s_pc], f32, name="sinS", tag="sinS")
        sync.dma_start(cosS, cos_d)
        sync.dma_start(sinS, sin_d)
        ones1 = constp.tile([1, 128], f32, name="ones1", tag="ones1")
        vec.memset(ones1, 1.0)
        scal_sb = constp.tile([128, 4], f32, name="scal_sb", tag="scal_sb")
        scal_row = constp.tile([1, 4], f32, name="scal_row", tag="scal_row")
        sync.dma_start(scal_row, sc_d)
        sw_q, sw_k, sw_v, sw_o = (scal_sb[:, i:i + 1] for i in range(4))
        ones_sb = constp.tile([128, 1], f32r, name="ones_sb", tag="ones_sb")
        sync.dma_start(ones_sb, onesr_d)

        # quantized+transposed activations [hid, tok] as bf16 integers
        xT = xTp.tile([128, HB, s_pc], bf16, name="xT", tag="xT")
        # per-token dequant scale r_i = clip(absmax,1e-5)/127, replicated on all partitions
        R = xTp.tile([128, s_pc], f32, name="R", tag="R")
        r_dram = dramp.tile([1, s_pc], f32, name="r_dram", tag="r_dram")

        qTall = qTp.tile([128, QH, s_pc], f32r, name="qTall", tag="qTall")
        aoall = aop.tile([128, QH, s_pc], f32, name="aoall", tag="aoall")
        acc = aop.tile([128, s_pc], f32, name="acc", tag="acc")
        vec.memset(acc, 0.0)

        # collective buffers
        ksrc = dramp.tile([KVH, D, s_pc], f32r, name="ksrc", tag="ksrc")
        vsrc = dramp.tile([NT, 128, KVH * D], f32r, name="vsrc", tag="vsrc")
        kv_space = "Local" if stub_collectives else "Shared"
        KG = dramp.tile([n_cores, KVH, D, s_pc], f32r, name="KG", tag="KG",
                        addr_space=kv_space)
        VG = dramp.tile([n_cores, NT, 128, KVH * D], f32r, name="VG", tag="VG",
                        addr_space=kv_space)

        r_tiles = []

        # per-token scale tiles (partition layout) -- live into phase 2
        for t in range(NT):
            r_t = constp.tile([128, 1], f32, name=f"r_{t}", tag=f"r_{t}")
            r_tiles.append(r_t)

        # ---------------- phase 1: layernorm + act quant ----------------
        with tc.tile_pool(name=pfx + "lnp", bufs=1) as lnp, \
             tc.tile_pool(name=pfx + "gbp", bufs=1) as gbp, \
             tc.tile_pool(name=pfx + "statp", bufs=1) as statp, \
             tc.tile_pool(name=pfx + "psb1", bufs=2, space="PSUM") as psb1, \
             tc.tile_pool(name=pfx + "xqp", bufs=2) as xqp:
            if not skip_gb:
                Gt = gbp.tile([128, H], f32, name="Gt", tag="Gt")
                Bt = gbp.tile([128, H], f32, name="Bt", tag="Bt")
                grow = gbp.tile([1, H], f32, name="grow", tag="grow")
                brow = gbp.tile([1, H], f32, name="brow", tag="brow")
                sync.dma_start(grow, g_d)
                sync.dma_start(brow, b_d)
                bcast_row(psb1, ones1, grow, Gt, H, "g")
                bcast_row(psb1, ones1, brow, Bt, H, "b")
            bcast_row(psb1, ones1, scal_row, scal_sb, 4, "sc")

            for t in range(NT):
                xs = lnp.tile([128, H], f32, name=f"xs{t}", tag="xs")
                scr = lnp.tile([128, H], f32, name=f"scr{t}", tag="scr")
                sync.dma_start(xs, x_d[t * 128:(t + 1) * 128, :])

                nsum = statp.tile([128, 1], f32, name=f"nsum{t}", tag=f"nsum{t}")
                vec.tensor_reduce(nsum, xs, axis=AX.X, op=ALU.add, negate=True)
                nmu = statp.tile([128, 1], f32, name=f"nmu{t}", tag=f"nmu{t}")
                vec.tensor_scalar_mul(nmu, nsum, 1.0 / H)
                sumsq = statp.tile([128, 1], f32, name=f"sumsq{t}", tag=f"sumsq{t}")
                act.activation(scr, xs, AF.Square, bias=nmu, scale=1.0,
                               accum_out=sumsq)
                varv = statp.tile([128, 1], f32, name=f"varv{t}", tag=f"varv{t}")
                vec.tensor_scalar(varv, sumsq, 1.0 / H, LN_EPS, ALU.mult, ALU.add)
                stdv = statp.tile([128, 1], f32, name=f"stdv{t}", tag=f"stdv{t}")
                act.activation(stdv, varv, AF.Sqrt)
                rstd = statp.tile([128, 1], f32, name=f"rstd{t}", tag=f"rstd{t}")
                vec.reciprocal(rstd, stdv)
                nmr = statp.tile([128, 1], f32, name=f"nmr{t}", tag=f"nmr{t}")
                vec.tensor_mul(nmr, nmu, rstd)
                # normed = x*rstd + (-mu*rstd), then *g + b (in place)
                act.activation(xs, xs, AF.Identity, bias=nmr, scale=rstd)
                if not skip_gb:
                    vec.tensor_mul(xs, xs, Gt)
                    vec.tensor_add(xs, xs, Bt)

                am = statp.tile([128, 1], f32, name=f"am{t}", tag=f"am{t}")
                vec.tensor_reduce(am, xs, axis=AX.X, op=ALU.max,
                                  apply_absolute_value=True)
                amc = statp.tile([128, 1], f32, name=f"amc{t}", tag=f"amc{t}")
                vec.tensor_scalar_max(amc, am, 1e-5)
                r_t = r_tiles[t]
                vec.tensor_scalar_mul(r_t, amc, 1.0 / QB)
                inv = statp.tile([128, 1], f32, name=f"inv{t}", tag=f"inv{t}")
                vec.reciprocal(inv, amc)
                scq = statp.tile([128, 1], f32, name=f"scq{t}", tag=f"scq{t}")
                vec.tensor_scalar_mul(scq, inv, QB)

                # n = round(normed * scq), exact via +C trick; write as bf16 ints
                vec.tensor_scalar(scr, xs, scq, ROUND_C, ALU.mult, ALU.add)
                xq = xqp.tile([128, H], bf16, name=f"xq{t}", tag="xq")
                vec.tensor_scalar_add(xq, scr, -ROUND_C)

                # transpose into [hid, tok] layout (DMA xbar transpose, bf16)
                for h in range(HB):
                    sync.dma_start(xT[:, h, t * 128:(t + 1) * 128],
                                   xq[:, h * 128:(h + 1) * 128], transpose=True)
                # export per-token scale
                sync.dma_start(r_dram[0, t * 128:(t + 1) * 128], r_t[:, 0])

            r_row = constp.tile([1, s_pc], f32, name="r_row", tag="r_row")
            sync.dma_start(r_row, r_dram[:])
            bcast_row(psb1, ones1, r_row, R, s_pc, "r")

        # ---------------- phase 2: K,V projections + rope + gather ----------------
        with tc.tile_pool(name=pfx + "wkvp", bufs=3) as wkvp, \
             tc.tile_pool(name=pfx + "pskv", bufs=2, space="PSUM") as pskv, \
             tc.tile_pool(name=pfx + "psv", bufs=1, space="PSUM") as psvp, \
             tc.tile_pool(name=pfx + "kdrp", bufs=2) as kdrp:
            # K projection: kT[feat, tok] per kv head
            for f in range(KVH):
                wk_sb = wkvp.tile([128, HB, 128], bf16, name=f"wk{f}", tag="wkv")
                sync.dma_start(wk_sb, wk_d[:, f, :, :])
                ps = pskv.tile([128, s_pc], f32, name=f"psk{f}", tag="pskv")
                for k in range(HB):
                    pe.matmul(ps, wk_sb[:, k, :], xT[:, k, :],
                              start=(k == 0), stop=(k == HB - 1))
                kdr = kdrp.tile([128, s_pc], f32, name=f"kdr{f}", tag="kdr")
                vec.scalar_tensor_tensor(kdr, ps, sw_k, R, op0=ALU.mult,
                                         op1=ALU.mult)
                # rope
                rot = kdrp.tile([128, s_pc], f32, name=f"krot{f}", tag="krot")
                sync.dma_start(rot[0:64, :], kdr[64:128, :])
                sync.dma_start(rot[64:128, :], kdr[0:64, :])
                vec.tensor_mul(rot, rot, sinS)
                kcos = kdrp.tile([128, s_pc], f32, name=f"kcos{f}", tag="kcos")
                vec.tensor_mul(kcos, kdr, cosS)
                krp = kdrp.tile([128, s_pc], f32r, name=f"krp{f}", tag="krp")
                vec.tensor_add(krp, kcos, rot)
                sync.dma_start(ksrc[f], krp)

            # V projection: v[tok, feat] (x^T as stationary, weights moving)
            psvs = [psvp.tile([128, KVH * D], f32, name=f"psv{t}",
                              tag=f"psv{t}") for t in range(NT)]
            for k in range(HB):
                wv_sb = wkvp.tile([128, KVH * D], bf16, name=f"wv{k}",
                                  tag="wv")
                sync.dma_start(wv_sb, wv_d[:, k, :])
                for t in range(NT):
                    for n0 in (0, 512):
                        pe.matmul(psvs[t][:, n0:n0 + 512],
                                  xT[:, k, t * 128:(t + 1) * 128],
                                  wv_sb[:, n0:n0 + 512],
                                  start=(k == 0), stop=(k == HB - 1))
            for t in range(NT):
                vdr = kdrp.tile([128, KVH * D], f32r, name=f"vdr{t}", tag="vdr")
                vec.tensor_scalar(vdr, psvs[t], r_tiles[t], sw_v,
                                  ALU.mult, ALU.mult)
                sync.dma_start(vsrc[t], vdr)

            if stub_collectives:
                for r in range(n_cores):
                    sync.dma_start(KG[r], ksrc)
                    sync.dma_start(VG[r], vsrc)
            else:
                gp.collective_compute(
                    "AllGather", ALU.bypass,
                    replica_groups=[list(range(n_cores))],
                    ins=[ksrc.opt()], outs=[KG.opt()])
                gp.collective_compute(
                    "AllGather", ALU.bypass,
                    replica_groups=[list(range(n_cores))],
                    ins=[vsrc.opt()], outs=[VG.opt()])

        # ---------------- phase 3: Q projection + rope ----------------
        with tc.tile_pool(name=pfx + "wqp", bufs=3) as wqp, \
             tc.tile_pool(name=pfx + "psq", bufs=4, space="PSUM") as psqp, \
             tc.tile_pool(name=pfx + "qdrp", bufs=2) as qdrp:
            for f in range(QH):
                wq_sb = wqp.tile([128, HB, 128], bf16, name=f"wq{f}", tag="wq")
                sync.dma_start(wq_sb, wq_d[:, f, :, :])
                ps = psqp.tile([128, s_pc], f32, name=f"psq{f}", tag="psq")
                for k in range(HB):
                    pe.matmul(ps, wq_sb[:, k, :], xT[:, k, :],
                              start=(k == 0), stop=(k == HB - 1))
                qdr = qdrp.tile([128, s_pc], f32, name=f"qdr{f}", tag="qdr")
                vec.scalar_tensor_tensor(qdr, ps, sw_q, R, op0=ALU.mult,
                                         op1=ALU.mult)
                rot = qdrp.tile([128, s_pc], f32, name=f"qrot{f}", tag="qrot")
                sync.dma_start(rot[0:64, :], qdr[64:128, :])
                sync.dma_start(rot[64:128, :], qdr[0:64, :])
                vec.tensor_mul(rot, rot, sinS)
                qcos = qdrp.tile([128, s_pc], f32, name=f"qcos{f}", tag="qcos")
                vec.tensor_mul(qcos, qdr, cosS)
                vec.tensor_add(qTall[:, f, :], qcos, rot)

        # ---------------- phase 4: attention ----------------
        with tc.tile_pool(name=pfx + "kgp", bufs=2) as kgp, \
             tc.tile_pool(name=pfx + "vgp", bufs=2) as vgp, \
             tc.tile_pool(name=pfx + "ep", bufs=2) as ep, \
             tc.tile_pool(name=pfx + "pss", bufs=2, space="PSUM") as pssp, \
             tc.tile_pool(name=pfx + "psd", bufs=1, space="PSUM") as psdp, \
             tc.tile_pool(name=pfx + "pso", bufs=2, space="PSUM") as psop, \
             tc.tile_pool(name=pfx + "psb4", bufs=1, space="PSUM") as psb4, \
             tc.tile_pool(name=pfx + "dnp", bufs=2) as dnp:
            for g in range(KVH):
                kg_sb = kgp.tile([128, n_cores, s_pc], f32r, name=f"kg{g}",
                                 tag="kg")
                sync.dma_start(kg_sb, KG[:, g, :, :].rearrange("r d t -> d r t"))
                vg_sb = vgp.tile([128, KT, 128], f32r, name=f"vg{g}", tag="vg")
                sync.dma_start(
                    vg_sb,
                    VG.rearrange("r t p f -> p (r t) f")[:, :,
                                                         g * 128:(g + 1) * 128])
                for j in range(NREP):
                    hq = g * NREP + j
                    qh_ap = qTall[:, hq, :]
                    eT = ep.tile([128, KT, s_pc], f32r, name=f"eT{g}_{j}",
                                 tag="eT")
                    # scores in groups of 4 key-tiles per PSUM tile so each
                    # Exp covers [128, 1024] (amortizes PSUM access latency)
                    GRP = 4
                    for kt0 in range(0, KT, GRP):
                        pss = pssp.tile([128, GRP * s_pc], f32,
                                        name=f"pss{g}_{j}_{kt0}", tag="pss")
                        for u in range(GRP):
                            kt = kt0 + u
                            pe.matmul(pss[:, u * s_pc:(u + 1) * s_pc],
                                      kg_sb[:, kt // NT,
                                            (kt % NT) * 128:
                                            (kt % NT) * 128 + 128],
                                      qh_ap, start=True, stop=True)
                        act.activation(
                            eT[:, kt0:kt0 + GRP, :].rearrange("p a b -> p (a b)"),
                            pss, AF.Exp, scale=SM_SCALE)
                    # denominators: ones^T @ eT accumulated over key tiles
                    psd = psdp.tile([1, s_pc], f32, name=f"psd{g}_{j}",
                                    tag="psd")
                    for kt in range(KT):
                        pe.matmul(psd, ones_sb,
                                  eT[:, kt, :],
                                  start=(kt == 0), stop=(kt == KT - 1))
                    # attention output (unnormalized): v^T as stationary
                    pso = psop.tile([128, s_pc], f32, name=f"pso{g}_{j}",
                                    tag="pso")
                    for kt in range(KT):
                        pe.matmul(pso, vg_sb[:, kt, :],
                                  eT[:, kt, :],
                                  start=(kt == 0), stop=(kt == KT - 1))
                    # normalize by softmax denominator
                    dnrow = dnp.tile([1, s_pc], f32, name=f"dnrow{g}_{j}",
                                     tag="dnrow")
                    vec.tensor_copy(dnrow, psd)
                    dps = psb4.tile([128, s_pc], f32, name=f"dps{g}_{j}",
                                    tag="dps")
                    pe.matmul(dps, ones1, dnrow, start=True, stop=True)
                    dnr = dnp.tile([128, s_pc], f32, name=f"dnr{g}_{j}",
                                   tag="dnr")
                    vec.reciprocal(dnr, dps)
                    vec.tensor_tensor(aoall[:, hq, :], pso, dnr, ALU.mult)
                    vec.tensor_tensor(acc, acc, aoall[:, hq, :], ALU.max)
                    vec.scalar_tensor_tensor(acc, aoall[:, hq, :], -1.0, acc,
                                             op0=ALU.mult, op1=ALU.max)

        # ---------------- phase 5: re-quant + O projection ----------------
        with tc.tile_pool(name=pfx + "q2p", bufs=1) as q2p, \
             tc.tile_pool(name=pfx + "t1p", bufs=2) as t1p, \
             tc.tile_pool(name=pfx + "wop", bufs=3) as wop, \
             tc.tile_pool(name=pfx + "psy", bufs=4, space="PSUM") as psyp, \
             tc.tile_pool(name=pfx + "psb5", bufs=1, space="PSUM") as psb5, \
             tc.tile_pool(name=pfx + "yp", bufs=2) as yp:
            # per-token absmax over the partition dim: halving tree
            # (acc is already elementwise |.|-accumulated, all >= 0)
            tmp = q2p.tile([64, s_pc], f32, name="tmphalf", tag="tmphalf")
            cur = 128
            while cur > 1:
                h = cur // 2
                sync.dma_start(tmp[0:h, :], acc[h:cur, :])
                vec.tensor_tensor(acc[0:h, :], acc[0:h, :], tmp[0:h, :],
                                  ALU.max)
                cur = h
            r2row = q2p.tile([1, s_pc], f32, name="r2row", tag="r2row")
            vec.tensor_scalar(r2row, acc[0:1, :], 1e-5, 1.0 / QB,
                              ALU.max, ALU.mult)
            R2 = q2p.tile([128, s_pc], f32, name="R2", tag="R2")
            bcast_row(psb5, ones1, r2row, R2, s_pc, "r2")
            sc2 = q2p.tile([128, s_pc], f32, name="sc2", tag="sc2")
            vec.reciprocal(sc2, R2)
            ao2 = q2p.tile([128, HB, s_pc], bf16, name="ao2", tag="ao2")
            for f in range(QH):
                t1 = t1p.tile([128, s_pc], f32, name=f"t1_{f}", tag="t1")
                vec.tensor_tensor(t1, aoall[:, f, :], sc2, ALU.mult)
                vec.tensor_scalar(ao2[:, f, :], t1, ROUND_C, -ROUND_C,
                                  ALU.add, ALU.add)

            for f in range(HB):
                wo_sb = wop.tile([128, HB, 128], bf16, name=f"wo{f}", tag="wo")
                sync.dma_start(wo_sb, wo_d[:, f, :, :])
                ps = psyp.tile([128, s_pc], f32, name=f"psy{f}", tag="psy")
                for k in range(HB):
                    pe.matmul(ps, wo_sb[:, k, :], ao2[:, k, :],
                              start=(k == 0), stop=(k == HB - 1))
                yT_sb = yp.tile([128, s_pc], f32, name=f"yT{f}", tag="yT")
                vec.scalar_tensor_tensor(yT_sb, ps, sw_o, R2, op0=ALU.mult,
                                         op1=ALU.mult)
                sync.dma_start(y_d[f * 128:(f + 1) * 128, :], yT_sb)


# ---------------------------------------------------------------------------
# host side
# ---------------------------------------------------------------------------

def _weight_quant_host(W):
    """Mimic reference _weight_quant: returns ternary m in {-1,0,1} and the
    effective dequant scale (1/scale) as fp32."""
    W = np.asarray(W, dtype=np.float32)
    mean_abs = np.float32(np.mean(np.abs(W), dtype=np.float64))
    clipped = np.maximum(mean_abs, np.float32(1e-5))
    scale = np.float32(1.0) / clipped
    m = np.clip(np.round(W * scale), -1.0, 1.0).astype(np.float32)
    sw = np.float32(1.0) / scale    # dequant scale applied after int matmul
    return m, float(sw)


def _prep_weights(Wq, Wk, Wv, Wo):
    mq, swq = _weight_quant_host(Wq)
    mk, swk = _weight_quant_host(Wk)
    mv, swv = _weight_quant_host(Wv)
    mo, swo = _weight_quant_host(Wo)

    def blocked(mT, fb):  # mT: [H, out] -> [128, fb, HB, 128]
        return np.ascontiguousarray(
            mT.reshape(HB, 128, fb, 128).transpose(1, 2, 0, 3)
        ).astype(np.float32)

    wqt = blocked(mq.T, QH)
    wkt = blocked(mk.T, KVH)
    wot = blocked(mo.T, HB)
    wvt = np.ascontiguousarray(
        mv.T.reshape(HB, 128, KVH * D).transpose(1, 0, 2)).astype(np.float32)
    import ml_dtypes
    tob = lambda a: a.astype(ml_dtypes.bfloat16)
    return (tob(wqt), tob(wkt), tob(wvt), tob(wot),
            np.array([[swq, swk, swv, swo]], dtype=np.float32))


def _rope_tables(S):
    inv = (1.0 / (10000.0 ** (np.arange(0, D, 2, dtype=np.float32)
                              / np.float32(D)))).astype(np.float32)
    pos = np.arange(S, dtype=np.float32)
    fr = pos[:, None] * inv[None, :]          # [S, 64]
    emb = np.concatenate([fr, fr], axis=1)    # [S, D]
    cosT = np.cos(emb).T.astype(np.float32).copy()   # [D, S]
    sinT = np.sin(emb).T.astype(np.float32).copy()
    sinT[0:64, :] *= -1.0                      # sign baked for rotate-half
    return cosT, sinT


def _in_maps(inputs, n_cores=8, s_pc=256):
    hs = np.asarray(inputs["hidden_states"], dtype=np.float32)
    ln_w = np.asarray(inputs["ln_w"], dtype=np.float32).reshape(1, H)
    ln_b = np.asarray(inputs["ln_b"], dtype=np.float32).reshape(1, H)
    wqt, wkt, wvt, wot, wscal = _prep_weights(
        inputs["Wq"], inputs["Wk"], inputs["Wv"], inputs["Wo"])
    S = hs.shape[1]
    cosT, sinT = _rope_tables(S)
    maps = []
    for c in range(n_cores):
        sl = slice(c * s_pc, (c + 1) * s_pc)
        maps.append({
            "x": np.ascontiguousarray(hs[0, sl, :]),
            "lng": ln_w, "lnb": ln_b,
            "cosT": np.ascontiguousarray(cosT[:, sl]),
            "sinTs": np.ascontiguousarray(sinT[:, sl]),
            "wqt": wqt, "wkt": wkt, "wvt": wvt, "wot": wot,
            "wscal": wscal,
            "onesr": np.ones((128, 1), dtype=np.float32),
        })
    return maps


_CACHED = {}


def _run(inputs, trace=False, n_cores=8, s_pc=256):
    from concourse.bass_utils import run_bass_kernel_spmd
    skip_gb = bool(
        np.allclose(np.asarray(inputs["ln_w"]), 1.0)
        and np.allclose(np.asarray(inputs["ln_b"]), 0.0))
    key = (n_cores, s_pc, skip_gb)
    if key not in _CACHED:
        _CACHED[key] = build(n_cores, s_pc, skip_gb=skip_gb)
    nc = _CACHED[key]
    maps = _in_maps(inputs, n_cores, s_pc)
    res = run_bass_kernel_spmd(nc, maps, list(range(n_cores)), trace=trace)
    parts = [res.results[c]["yT"].T for c in range(n_cores)]
    y = np.concatenate(parts, axis=0)[None, :, :].astype(np.float32)
    return y, res.exec_time_ns


def kernel(**inputs):
    y, _ = _run(inputs, trace=False)
    return y
